# revision 1
# baseline (speedup 1.0000x reference)
"""CompressedLinear (quantized low-rank linear) on 8 trn2 NeuronCores.

y = ((x @ dequant(Vh).T) * dequant(S)) @ dequant(U).T + bias

Strategy: data-parallel over tokens. x [4,2048,4096] -> 8192 tokens -> 1024
tokens/core. Weights replicated. Per core, two chained bf16 matmuls with
fp32 PSUM accumulation:

  mm1: hT[r, tok]  = (Vh_int - zp_v).T-tile.T @ xT-tile   (contract in_f)
  mm2: y[tok, o]   = hT-tile.T @ (U_int - zp_u).T-tile    (contract rank)

All multiplicative scales (Vh_scale * S_scale * U_scale) and the dequantized
S vector are folded into the hT eviction (per-partition scalar on rank), so
the integer-valued weights stay EXACT in bf16 (-128..127 fits in 8-bit
mantissa); the only bf16 rounding is on x and hT.

Host does pure layout work only: x transpose/shard, weight transpose + a
lossless int32->bf16 cast (values 0..255). All arithmetic (zero-point
subtract, scales, matmuls, bias) runs on device.
"""

import os

import numpy as np
import ml_dtypes

IN_F, OUT_F, RANK = 4096, 4096, 1024
B, S_LEN = 4, 2048
N_CORES = 8
P = 128
NTOK = B * S_LEN            # 8192 tokens total
TPC = NTOK // N_CORES       # 1024 tokens per core
TBS = 512                   # tokens per block (matmul moving free dim)
TB = TPC // TBS             # 2 token blocks per core
KO = IN_F // P              # 32 k-tiles (mm1 contraction)
RO = RANK // P              # 8 r-tiles (mm2 contraction / mm1 output)
NOB = OUT_F // 512          # 8 output-feature blocks of 512

_BF16 = ml_dtypes.bfloat16

# Set by kernel() for the benefit of test harnesses (exec time inspection).
last_run = None


def _build_nc(zp_v: float, zp_u: float, zp_s: float, s_mult: float):
    import concourse.mybir as mybir
    import concourse.tile as tile
    from concourse import bacc

    f32 = mybir.dt.float32
    bf16 = mybir.dt.bfloat16
    OP = mybir.AluOpType

    nc = bacc.Bacc("TRN2", target_bir_lowering=False, debug=False,
                   num_devices=N_CORES)

    xT = nc.dram_tensor("xT", [IN_F, TPC], f32, kind="ExternalInput")
    vhT = nc.dram_tensor("vhT", [IN_F, RANK], bf16, kind="ExternalInput")
    uT = nc.dram_tensor("uT", [RANK, OUT_F], bf16, kind="ExternalInput")
    sv = nc.dram_tensor("sv", [RANK], bf16, kind="ExternalInput")
    bias = nc.dram_tensor("bias", [OUT_F], f32, kind="ExternalInput")
    y = nc.dram_tensor("y", [TPC, OUT_F], f32, kind="ExternalOutput")

    with tile.TileContext(nc) as tc:
        with (
            tc.tile_pool(name="const", bufs=1) as const,
            tc.tile_pool(name="xin", bufs=3) as xin,
            tc.tile_pool(name="xbp", bufs=1) as xbp,
            tc.tile_pool(name="hTp", bufs=2) as hTp,
            tc.tile_pool(name="yout", bufs=4) as yout,
            tc.tile_pool(name="ps1", bufs=1, space="PSUM") as ps1,
            tc.tile_pool(name="ps2", bufs=4, space="PSUM") as ps2,
        ):
            # S vector -> folded per-rank scale: (S - zp_s) * (s_v*s_s*s_u)
            s_sb = const.tile([P, RO], bf16, name="s_sb")
            nc.sync.dma_start(s_sb[:], sv.ap().rearrange("(ro p) -> p ro", p=P))
            s_comb = const.tile([P, RO], f32, name="s_comb")
            nc.vector.tensor_scalar(s_comb[:], s_sb[:], zp_s, s_mult,
                                    OP.subtract, OP.mult)

            # bias replicated across all 128 partitions (tokens)
            bias_sb = const.tile([P, OUT_F], f32, name="bias_sb")
            nc.sync.dma_start(bias_sb[:],
                              bias.ap()[None, :].to_broadcast((P, OUT_F)))

            # Vh.T resident in SBUF: [128(k-in), 32(k-out), 1024(r)], bf16.
            # Subtract zero point in place (integers stay exact in bf16).
            vh_sb = const.tile([P, KO, RANK], bf16, name="vh_sb")
            vh_src = vhT.ap().rearrange("(ko p) r -> p ko r", p=P)
            for ko in range(KO):
                nc.sync.dma_start(vh_sb[:, ko, :], vh_src[:, ko, :])
                nc.vector.tensor_scalar(vh_sb[:, ko, :], vh_sb[:, ko, :],
                                        zp_v, None, OP.subtract)

            # U.T resident in SBUF: [128(r-in), 8(r-out), 4096(o)], bf16.
            u_sb = const.tile([P, RO, OUT_F], bf16, name="u_sb")
            u_src = uT.ap().rearrange("(ro p) o -> p ro o", p=P)
            for ro in range(RO):
                nc.sync.dma_start(u_sb[:, ro, :], u_src[:, ro, :])
                nc.vector.tensor_scalar(u_sb[:, ro, :], u_sb[:, ro, :],
                                        zp_u, None, OP.subtract)

            for blk in range(TB):
                tok0 = blk * TBS
                # ---- load + cast this block's x.T slice to bf16 ----
                xb = xbp.tile([P, KO, TBS], bf16, name="xb")
                for ko in range(KO):
                    xf = xin.tile([P, TBS], f32, name="xf")
                    nc.sync.dma_start(
                        xf[:], xT.ap()[ko * P:(ko + 1) * P, tok0:tok0 + TBS])
                    nc.scalar.copy(xb[:, ko, :], xf[:])

                # ---- mm1: hT[r, tok] over 2 halves of r (4 PSUM banks) ----
                hT = hTp.tile([P, RO, TBS], bf16, name="hT")
                for rh in range(2):
                    pst = [ps1.tile([P, TBS], f32, name=f"ps1_{ri}")
                           for ri in range(4)]
                    for ko in range(KO):
                        for ri in range(4):
                            rt = rh * 4 + ri
                            nc.tensor.matmul(
                                pst[ri][:],
                                vh_sb[:, ko, rt * P:(rt + 1) * P],
                                xb[:, ko, :],
                                start=(ko == 0), stop=(ko == KO - 1))
                    for ri in range(4):
                        rt = rh * 4 + ri
                        # hT = psum * s_comb[r]  (per-partition scalar)
                        nc.vector.tensor_tensor(
                            hT[:, rt, :], pst[ri][:],
                            s_comb[:, rt:rt + 1].to_broadcast((P, TBS)),
                            OP.mult)

                # ---- mm2: y[tok, o] ----
                for t in range(TBS // P):           # 4 token sub-tiles
                    for ob in range(NOB):           # 8 blocks of 512 outputs
                        psy = ps2.tile([P, 512], f32, name="ps2")
                        for rk in range(RO):
                            nc.tensor.matmul(
                                psy[:],
                                hT[:, rk, t * P:(t + 1) * P],
                                u_sb[:, rk, ob * 512:(ob + 1) * 512],
                                start=(rk == 0), stop=(rk == RO - 1))
                        yt = yout.tile([P, 512], f32, name="yt")
                        nc.vector.tensor_tensor(
                            yt[:], psy[:], bias_sb[:, ob * 512:(ob + 1) * 512],
                            OP.add)
                        r0 = tok0 + t * P
                        nc.sync.dma_start(
                            y.ap()[r0:r0 + P, ob * 512:(ob + 1) * 512], yt[:])

    nc.compile()
    return nc


def _maybe_enable_trace():
    """Register the axon NTFF profile hook (test/dev only, KERNEL_TRACE=1)."""
    try:
        import sys
        import types

        try:
            from antenv.axon_hooks import get_axon_ntff_profile_hook  # noqa: F401
        except ImportError:
            store = {"h": None}
            mod = types.ModuleType("antenv.axon_hooks")
            mod.set_axon_ntff_profile_hook = lambda h: store.__setitem__("h", h)
            mod.get_axon_ntff_profile_hook = lambda: store["h"]
            sys.modules["antenv.axon_hooks"] = mod
        from antenv.axon_hooks import set_axon_ntff_profile_hook
        from trn_agent_boot.trn_boot import _ntff_profile_via_ctypes

        set_axon_ntff_profile_hook(
            _ntff_profile_via_ctypes("/opt/axon/libaxon_pjrt.so"))
        import concourse.bass_utils as bass_utils

        bass_utils.upload_artifacts = lambda tmpdir: tmpdir
        return True
    except Exception as e:  # pragma: no cover - trace is best-effort
        print(f"trace setup failed: {e}")
        return False


def kernel(x, U_data, U_scale, U_zp, S_data, S_scale, S_zp,
           Vh_data, Vh_scale, Vh_zp, bias):
    global last_run

    trace = bool(os.environ.get("KERNEL_TRACE"))
    if trace:
        trace = _maybe_enable_trace()

    from concourse.bass_utils import run_bass_kernel_spmd

    x = np.asarray(x, dtype=np.float32)
    bias_np = np.asarray(bias, dtype=np.float32)
    s_v = float(np.asarray(Vh_scale).reshape(-1)[0])
    s_u = float(np.asarray(U_scale).reshape(-1)[0])
    s_s = float(np.asarray(S_scale).reshape(-1)[0])
    zp_v = float(np.asarray(Vh_zp).reshape(-1)[0])
    zp_u = float(np.asarray(U_zp).reshape(-1)[0])
    zp_s = float(np.asarray(S_zp).reshape(-1)[0])

    # Pure layout work on host (no arithmetic): transpose + lossless casts.
    xT = np.ascontiguousarray(x.reshape(NTOK, IN_F).T)           # [4096, 8192]
    vhT = np.ascontiguousarray(
        np.asarray(Vh_data).T).astype(_BF16)                     # [4096, 1024]
    uT = np.ascontiguousarray(np.asarray(U_data).T).astype(_BF16)  # [1024, 4096]
    sv = np.asarray(S_data).astype(_BF16)                        # [1024]

    nc = _build_nc(zp_v, zp_u, zp_s, s_v * s_s * s_u)

    in_maps = []
    for c in range(N_CORES):
        in_maps.append({
            "xT": np.ascontiguousarray(xT[:, c * TPC:(c + 1) * TPC]),
            "vhT": vhT,
            "uT": uT,
            "sv": sv,
            "bias": bias_np,
        })

    kwargs = {}
    if trace:
        kwargs = dict(trace=True, tmpdir=os.environ.get("KERNEL_TRACE_DIR"))
    res = run_bass_kernel_spmd(nc, in_maps, core_ids=list(range(N_CORES)),
                               **kwargs)
    last_run = res

    y = np.concatenate([res.results[c]["y"] for c in range(N_CORES)], axis=0)
    return y.reshape(B, S_LEN, OUT_F)
